# revision 18
# baseline (speedup 1.0000x reference)
"""Multi-head attention (N=2048, d_model=1024, H=16) on 8 trn2 cores.

Sharding: tensor-parallel over heads. Each core computes 2 heads (128 of the
1024 d_model dims): QKV projections for its head slice, scores + softmax + AV
for its 2 heads, and a partial output projection against its 128 rows of
Wo^T. Host sums the 8 partial outputs and adds bo.

v2 schedule: the Tile list-scheduler picks the lowest-priority READY op per
engine, so emission order is the schedule. The kernel emits a software
pipeline at per-m-tile granularity: [score-pair, exp, trailing AV-pairs, one
filler job], where filler jobs (K/Q/V projections, output projections of
earlier slices) are spread so the PE never runs a long projection burst that
starves the exp stream, and the input DMA order matches consumption order.

Layout (transposes host-side):
  - qT/kT/vT [1024, 2048] fed transposed; wqkv = [WqT|WkT|WvT] packed.
  - Q^T/K^T [128, 2048] (head dim on partitions); V' natural [m, 130]
    with ones columns at 64/129 so the AV matmul also emits the softmax
    denominator (psum row 64).
  - scores S^T per (m-tile, head) as c=64 row-group pairs (concurrent on
    the PE), exp straight out of psum on ACT (no max-subtraction: |s|/8 is
    within fp32-exp-safe range), AV accumulates [65, 512] per head.
  - normalization on-device: denominator reciprocal (DVE) broadcast via
    gpsimd mid-stream / via a c=1 PE matmul on the tail, multiply into OT.
  - y = OT^T @ woT per n-tile of 128 rows, shipped per-tile.
"""

import math
from collections import deque

import numpy as np
from ml_dtypes import bfloat16

N = 2048
D = 1024
H = 16
DK = 64
NCORES = 8
HPC = H // NCORES  # heads per core = 2
DL = HPC * DK  # local head dims per core = 128

NSL = 4  # n slices of 512
SL = 512
MT = 16  # m tiles of 128
CT = 8  # c tiles of 128

_CACHE = {}


def _build_nc(debug=False, with_bias=False):
    from contextlib import ExitStack

    import concourse.mybir as mybir
    import concourse.tile as tile
    from concourse import bacc

    f32 = mybir.dt.float32
    bf16 = mybir.dt.bfloat16
    AF = mybir.ActivationFunctionType

    nc = bacc.Bacc("TRN2", target_bir_lowering=False, debug=debug)

    qT = nc.dram_tensor("qT", [D, N], bf16, kind="ExternalInput")
    kT = nc.dram_tensor("kT", [D, N], bf16, kind="ExternalInput")
    vT = nc.dram_tensor("vT", [D, N], bf16, kind="ExternalInput")
    wqkv = nc.dram_tensor("wqkv", [D, 3 * DL], bf16, kind="ExternalInput")
    woT = nc.dram_tensor("woT", [DL, D], bf16, kind="ExternalInput")
    if with_bias:
        bq = nc.dram_tensor("bq", [DL, 1], f32, kind="ExternalInput")
        bk = nc.dram_tensor("bk", [DL, 1], f32, kind="ExternalInput")
        bvb = nc.dram_tensor("bvb", [128, DL], f32, kind="ExternalInput")
    y = nc.dram_tensor("y", [N, D], bf16, kind="ExternalOutput")

    kT_r = kT.rearrange("(t p) n -> p t n", p=128)
    qT_r = qT.rearrange("(t p) n -> p t n", p=128)
    vT_r = vT.rearrange("(t p) n -> p t n", p=128)
    w_r = wqkv.rearrange("(t p) d -> p t d", p=128)

    with tile.TileContext(nc) as tc, ExitStack() as ctx:
        const = ctx.enter_context(tc.tile_pool(name="const", bufs=1))
        xin = ctx.enter_context(tc.tile_pool(name="xin", bufs=1))
        acts = ctx.enter_context(tc.tile_pool(name="acts", bufs=1))
        ptp = ctx.enter_context(tc.tile_pool(name="ptp", bufs=18))
        ysp = ctx.enter_context(tc.tile_pool(name="ysp", bufs=4))
        smal = ctx.enter_context(tc.tile_pool(name="smal", bufs=3))
        # PSUM (8 banks): scores 2x2 (double-buffered pairs), av 2, ps 2
        spp = ctx.enter_context(tc.tile_pool(name="spp", bufs=2, space="PSUM"))
        avp = ctx.enter_context(tc.tile_pool(name="avp", bufs=1, space="PSUM"))
        psp = ctx.enter_context(tc.tile_pool(name="psp", bufs=2, space="PSUM"))

        # ---- persistent tiles ----
        w3_sb = const.tile([128, CT, 3 * DL], bf16, name="w3_sb")
        wo_sb = const.tile([128, D], bf16, name="wo_sb")
        wrm = const.tile([128, 128], bf16, name="wrm")
        ones_bf = const.tile([1, 64], bf16, name="ones_bf")
        wq_sb = w3_sb[:, :, 0:DL]
        wk_sb = w3_sb[:, :, DL : 2 * DL]
        wv_sb = w3_sb[:, :, 2 * DL : 3 * DL]

        kt_sb = xin.tile([128, CT, N], bf16, name="kt_sb")
        qt_sb = xin.tile([128, CT, N], bf16, name="qt_sb")
        vt_sb = xin.tile([128, CT, N], bf16, name="vt_sb")

        KT_sb = acts.tile([128, N], bf16, name="KT_sb")
        QT_sb = acts.tile([128, N], bf16, name="QT_sb")
        Vp_sb = acts.tile([128, MT, 130], bf16, name="Vp_sb")
        OT_sb = acts.tile([128, N], bf16, name="OT_sb")



        if with_bias:
            bq_dm = const.tile([DL, 1], f32, name="bq_dm")
            bk_dm = const.tile([DL, 1], f32, name="bk_dm")
            bvb_dm = const.tile([128, DL], f32, name="bvb_dm")
            bq_sb = const.tile([DL, 1], f32, name="bq_sb")
            bk_sb = const.tile([DL, 1], f32, name="bk_sb")
            bvb_sb = const.tile([128, DL], f32, name="bvb_sb")

        # per-use psum allocation: fresh tile handles from a bufs=2 pool so
        # Tile tracks each accumulation group independently (tile-granularity
        # dep tracking — a single shared tile would serialize everything)
        def next_ps():
            return psp.tile([128, SL], f32, name="ps", tag="ps")

        # ---- t=0: memsets + PE warmup (no DMA deps) ----
        nc.vector.memset(wrm, 1.0)
        nc.vector.memset(ones_bf, 1.0)
        nc.vector.memset(Vp_sb[:, :, 64:65], 1.0)
        nc.vector.memset(Vp_sb[:, :, 129:130], 1.0)
        warm_ps = next_ps()
        for i in range(20):
            nc.tensor.matmul(
                warm_ps[:, 0:128], lhsT=wrm, rhs=wrm, start=True, stop=True
            )
        # preload the exp table set during the DMA wait (saves ~2.7us of
        # ACT_TABLE_LOAD on the first-exp critical path)
        tbl_warm = smal.tile([1, 8], bf16, name="tbl_warm", tag="tblw")
        nc.scalar.activation(
            out=tbl_warm, in_=wrm[0:1, 0:8], func=AF.Exp, scale=1.0
        )

        # ---- input DMAs in consumption order ----
        nc.sync.dma_start(out=w3_sb[:, 0:4, :], in_=w_r[:, 0:4, :])
        nc.sync.dma_start(out=w3_sb[:, 4:8, :], in_=w_r[:, 4:8, :])
        if with_bias:
            nc.sync.dma_start(out=bq_dm, in_=bq[:, :])
            nc.sync.dma_start(out=bk_dm, in_=bk[:, :])
            nc.sync.dma_start(out=bvb_dm, in_=bvb[:, :])
            nc.vector.tensor_copy(out=bq_sb, in_=bq_dm)
            nc.vector.tensor_copy(out=bk_sb, in_=bk_dm)
            nc.vector.tensor_copy(out=bvb_sb, in_=bvb_dm)
        nc.sync.dma_start(out=kt_sb[:, :, 0:128], in_=kT_r[:, :, 0:128])
        nc.sync.dma_start(out=qt_sb[:, :, 0:512], in_=qT_r[:, :, 0:512])
        nc.sync.dma_start(out=kt_sb[:, :, 128:512], in_=kT_r[:, :, 128:512])
        nc.sync.dma_start(out=kt_sb[:, :, 512:1024], in_=kT_r[:, :, 512:1024])
        nc.sync.dma_start(out=vt_sb[:, :, 0:512], in_=vT_r[:, :, 0:512])
        nc.sync.dma_start(out=qt_sb[:, :, 512:1024], in_=qT_r[:, :, 512:1024])
        nc.sync.dma_start(out=kt_sb[:, :, 1024:1536], in_=kT_r[:, :, 1024:1536])
        nc.sync.dma_start(out=vt_sb[:, :, 512:1024], in_=vT_r[:, :, 512:1024])
        nc.sync.dma_start(out=kt_sb[:, :, 1536:2048], in_=kT_r[:, :, 1536:2048])
        nc.sync.dma_start(out=vt_sb[:, :, 1024:1536], in_=vT_r[:, :, 1024:1536])
        nc.sync.dma_start(out=vt_sb[:, :, 1536:2048], in_=vT_r[:, :, 1536:2048])
        nc.sync.dma_start(out=wo_sb, in_=woT[:, :])
        nc.sync.dma_start(out=qt_sb[:, :, 1024:1536], in_=qT_r[:, :, 1024:1536])
        nc.sync.dma_start(out=qt_sb[:, :, 1536:2048], in_=qT_r[:, :, 1536:2048])

        bqx = bq_sb if with_bias else None
        bkx = bk_sb if with_bias else None

        # ---- projection job helpers ----
        # Each job is atomic: it acquires a ps half at RUN time and finishes
        # its full accumulation group + evict before anyone else can claim
        # that half (next_ps rotation + Tile range deps serialize cleanly).
        def proj_kq(XT, wx, xt, bx, lo, hi):
            w = hi - lo
            ps = next_ps()
            for ct in range(CT):
                nc.tensor.matmul(
                    ps[:, 0:w],
                    lhsT=wx[:, ct, :],
                    rhs=xt[:, ct, lo:hi],
                    start=(ct == 0),
                    stop=(ct == CT - 1),
                )
            if bx is not None:
                nc.vector.tensor_scalar_add(
                    out=XT[:, lo:hi], in0=ps[:, 0:w], scalar1=bx
                )
            else:
                nc.vector.tensor_copy(out=XT[:, lo:hi], in_=ps[:, 0:w])

        def job_proj_kq(XT, wx, xt, bx, lo, hi):
            def run():
                proj_kq(XT, wx, xt, bx, lo, hi)

            return run

        def job_proj_v(mt):
            def run():
                ps = next_ps()
                for ct in range(CT):
                    nc.tensor.matmul(
                        ps[:, 0:128],
                        lhsT=vt_sb[:, ct, mt * 128 : (mt + 1) * 128],
                        rhs=wv_sb[:, ct, :],
                        start=(ct == 0),
                        stop=(ct == CT - 1),
                    )
                if with_bias:
                    nc.vector.tensor_add(
                        out=Vp_sb[:, mt, 0:64],
                        in0=ps[:, 0:64],
                        in1=bvb_sb[:, 0:64],
                    )
                    nc.vector.tensor_add(
                        out=Vp_sb[:, mt, 65:129],
                        in0=ps[:, 64:128],
                        in1=bvb_sb[:, 64:128],
                    )
                else:
                    nc.vector.tensor_copy(
                        out=Vp_sb[:, mt, :].rearrange("p (h e) -> p h e", h=2)[
                            :, :, 0:64
                        ],
                        in_=ps[:, 0:128].rearrange("p (h e) -> p h e", h=2),
                    )

            return run

        def job_yproj(nt, tail=False):
            def run():
                ysb = ysp.tile([128, D], bf16, name="ysb", tag="ysb")
                for chalf in range(2):
                    ps = next_ps()
                    nc.tensor.matmul(
                        ps,
                        lhsT=OT_sb[:, nt * 128 : (nt + 1) * 128],
                        rhs=wo_sb[:, chalf * SL : (chalf + 1) * SL],
                        start=True,
                        stop=True,
                    )
                    # tail: ACT is idle — split the evictions across engines
                    if tail and chalf == 1:
                        nc.scalar.copy(
                            out=ysb[:, chalf * SL : (chalf + 1) * SL], in_=ps
                        )
                    else:
                        nc.vector.tensor_copy(
                            out=ysb[:, chalf * SL : (chalf + 1) * SL], in_=ps
                        )
                nc.sync.dma_start(out=y[nt * 128 : (nt + 1) * 128, :], in_=ysb)

            return run

        # ---- head: first projections (K m0, Q slice 0) ----
        proj_kq(KT_sb, wk_sb, kt_sb, bkx, 0, 128)
        proj_kq(QT_sb, wq_sb, qt_sb, bqx, 0, 512)

        # ---- filler jobs: (gate_slot, closure) ----
        jobs = []
        jobs.append((0, job_proj_kq(KT_sb, wk_sb, kt_sb, bkx, 128, 512)))
        jobs.append((1, job_proj_kq(KT_sb, wk_sb, kt_sb, bkx, 512, 1024)))
        jobs.append((5, job_proj_kq(KT_sb, wk_sb, kt_sb, bkx, 1024, 1536)))
        jobs.append((10, job_proj_kq(KT_sb, wk_sb, kt_sb, bkx, 1536, 2048)))
        jobs.append((9, job_proj_kq(QT_sb, wq_sb, qt_sb, bqx, 512, 1024)))
        jobs.append((24, job_proj_kq(QT_sb, wq_sb, qt_sb, bqx, 1024, 1536)))
        jobs.append((40, job_proj_kq(QT_sb, wq_sb, qt_sb, bqx, 1536, 2048)))
        vslots = [3, 4, 7, 8, 12, 13, 14, 15, 16, 17, 18, 19, 20, 21, 22, 23]
        v_emit_slot = {}
        for mt in range(MT):
            v_emit_slot[mt] = vslots[mt]
            jobs.append((vslots[mt], job_proj_v(mt)))
        jobs = deque(sorted(jobs, key=lambda t: t[0]))

        yq = deque()  # yproj jobs gated on norm emission

        # ---- normalization ----
        def emit_norm(ns, h, av, tail=False):
            nsl = slice(ns * SL, (ns + 1) * SL)
            hd = slice(h * DK, (h + 1) * DK)
            den = smal.tile([1, SL], f32, name="den", tag="den")
            oc = smal.tile([64, SL], f32, name="oc", tag="oc")
            nc.vector.tensor_copy(out=den, in_=av[64:65, :])
            nc.vector.tensor_copy(out=oc, in_=av[0:64, :])
            rawr = smal.tile([1, SL], f32, name="rawr", tag="rawr")
            recip = smal.tile([1, SL], f32, name="recip", tag="recip")
            # custom-DVE op sandwiched between native DVE ops (in-order queue)
            nc.vector.reciprocal_approx_fast(out=rawr, in_=den)
            nc.vector.tensor_copy(out=recip, in_=rawr)
            if not tail:
                bc = smal.tile([64, SL], f32, name="bc", tag="bc")
                nc.gpsimd.partition_broadcast(out_ap=bc, in_ap=recip)
                nc.vector.tensor_mul(out=OT_sb[hd, nsl], in0=oc, in1=bc)
            else:
                # tail: broadcast via a c=1 matmul (PE idle, gpsimd slow)
                rb = smal.tile([1, SL], bf16, name="rb", tag="rb")
                nc.vector.tensor_copy(out=rb, in_=recip)
                bps = next_ps()
                nc.tensor.matmul(
                    bps[0:64, :], lhsT=ones_bf, rhs=rb, start=True, stop=True
                )
                nc.vector.tensor_mul(out=OT_sb[hd, nsl], in0=oc, in1=bps[0:64, :])

        # ---- main pipeline ----
        av_q = deque()
        for ns in range(NSL):
            for mt in range(MT):
                g = ns * MT + mt
                gate = v_emit_slot[mt] + 1 if ns == 0 else g + 4
                av_q.append((gate, ns, mt))
        n_av_done = [0] * NSL
        slice_avs = {}

        def emit_av(ns, mt, pt):
            if mt == 0:
                slice_avs[ns] = tuple(
                    avp.tile([65, SL], f32, name=f"av{h}", tag=f"av{h}")
                    for h in range(HPC)
                )
            avs = slice_avs[ns]
            for h in range(HPC):
                nc.tensor.matmul(
                    avs[h],
                    lhsT=Vp_sb[:, mt, 65 * h : 65 * h + 65],
                    rhs=pt[:, h, :],
                    start=(mt == 0),
                    stop=(mt == MT - 1),
                )
            n_av_done[ns] += 1
            if n_av_done[ns] == MT:
                tail = ns == NSL - 1
                for h in range(HPC):
                    emit_norm(ns, h, avs[h], tail=tail)
                gate = cur_g[0] + 1
                for nt in range(4 * ns, 4 * ns + 4):
                    yq.append((gate, job_yproj(nt, tail=tail)))

        pts = {}
        cur_g = [0]
        for ns in range(NSL):
            nsl = slice(ns * SL, (ns + 1) * SL)
            for mt in range(MT):
                g = ns * MT + mt
                cur_g[0] = g
                sp = spp.tile([128, HPC, SL], f32, name="sp", tag="sp")
                for h in range(HPC):
                    hd = slice(h * DK, (h + 1) * DK)
                    nc.tensor.matmul(
                        sp[:, h, :],
                        lhsT=KT_sb[hd, mt * 128 : (mt + 1) * 128],
                        rhs=QT_sb[hd, nsl],
                        start=True,
                        stop=True,
                    )
                pt = ptp.tile([128, HPC, SL], bf16, name="pt", tag="pt")
                nc.scalar.activation(
                    out=pt, in_=sp, func=AF.Exp, scale=1.0 / math.sqrt(DK)
                )
                pts[(ns, mt)] = pt
                # trailing AV pairs (up to 2 per slot)
                drained = 0
                while av_q and av_q[0][0] <= g and drained < 2:
                    _, ans, amt = av_q.popleft()
                    emit_av(ans, amt, pts.pop((ans, amt)))
                    drained += 1
                # one filler job per slot (lowest gate first across queues)
                jg = jobs[0][0] if jobs else 1 << 30
                yg = yq[0][0] if yq else 1 << 30
                if min(jg, yg) <= g:
                    if jg <= yg:
                        jobs.popleft()[1]()
                    else:
                        yq.popleft()[1]()

        # ---- drain ----
        while jobs:
            jobs.popleft()[1]()
        while av_q:
            _, ans, amt = av_q.popleft()
            emit_av(ans, amt, pts.pop((ans, amt)))
        while yq:
            yq.popleft()[1]()

    nc.finalize()
    return nc


def _get_nc(with_bias=False):
    key = ("nc", with_bias)
    if key not in _CACHE:
        _CACHE[key] = _build_nc(with_bias=with_bias)
    return _CACHE[key]


def _prepare_in_maps(q, k, v, Wq, bq, Wk, bk, Wv, bv, Wo, bo, with_bias=False):
    f32 = np.float32
    q = np.asarray(q, f32)
    k = np.asarray(k, f32)
    v = np.asarray(v, f32)
    Wq = np.asarray(Wq, f32)
    Wk = np.asarray(Wk, f32)
    Wv = np.asarray(Wv, f32)
    Wo = np.asarray(Wo, f32)
    qT = np.ascontiguousarray(q.T).astype(bfloat16)
    kT = np.ascontiguousarray(k.T).astype(bfloat16)
    vT = np.ascontiguousarray(v.T).astype(bfloat16)
    in_maps = []
    for i in range(NCORES):
        hs = slice(i * DL, (i + 1) * DL)
        wqkv = np.concatenate(
            [Wq[hs, :].T, Wk[hs, :].T, Wv[hs, :].T], axis=1
        )  # [1024, 384]
        m = {
            "qT": qT,
            "kT": kT,
            "vT": vT,
            "wqkv": np.ascontiguousarray(wqkv).astype(bfloat16),
            "woT": np.ascontiguousarray(Wo[:, hs].T).astype(bfloat16),
        }
        if with_bias:
            m["bq"] = np.ascontiguousarray(np.asarray(bq, f32)[hs].reshape(DL, 1))
            m["bk"] = np.ascontiguousarray(np.asarray(bk, f32)[hs].reshape(DL, 1))
            m["bvb"] = np.ascontiguousarray(
                np.broadcast_to(np.asarray(bv, f32)[hs], (128, DL))
            )
        in_maps.append(m)
    return in_maps


def kernel(q, k, v, Wq, bq, Wk, bk, Wv, bv, Wo, bo):
    from concourse.bass_utils import run_bass_kernel_spmd

    with_bias = bool(
        np.any(np.asarray(bq)) or np.any(np.asarray(bk)) or np.any(np.asarray(bv))
    )
    nc = _get_nc(with_bias=with_bias)
    in_maps = _prepare_in_maps(
        q, k, v, Wq, bq, Wk, bk, Wv, bv, Wo, bo, with_bias=with_bias
    )
    res = run_bass_kernel_spmd(nc, in_maps, core_ids=list(range(NCORES)))
    y = np.zeros((N, D), np.float32)
    for r in res.results:
        y += np.asarray(r["y"], np.float32)
    y += np.asarray(bo, np.float32)
    return y
